# revision 19
# baseline (speedup 1.0000x reference)
"""Trainium2 Bass kernel for nn_Bfly_BertIntermediate (butterfly MLP + bias + gelu).

Algorithm ("Monarch" factorization of the 10-layer butterfly over N=1024):
  layers 0..6  (strides 1..64) == block-diagonal A: 8 blocks of 128x128 per stack
  layers 7..9  (strides 128..512) == per-residue-128 mixing B: 8x8 over block idx j

v2 device pipeline per core (2048 tokens, data-parallel over 8 cores):
  x arrives feature-major f16 -> stage-A matmuls (feature-major psum out)
  -> DVE/Pool alternating cast-evac f32->f16 into y_ba (full-stack buffer)
  -> one 4-KiB-descriptor shuffle DMA per (stack, residue-group) y_ba->y_sa
  -> stage-B of-major matmuls (out = Bw.T @ y_sa, features on partitions)
  -> ACT gelu with fused per-partition bias, contiguous f16 writes
  -> 16-KiB-descriptor SWDGE stores to a permuted feature-major HBM layout
  (host unpermutes/transposes during unshard).

A/Bw/bias are a tiny host-side repacking of the twiddle weights (~0.1 GFLOP).
"""
import numpy as np

import concourse.bass as bass
import concourse.mybir as mybir
import concourse.tile as tile
from concourse import bacc, bass_utils

# problem shapes (hardcoded per harness contract)
B_, S_, N_ = 4, 4096, 1024
NSTACKS, LOG_N = 4, 10
SPLIT = 7                      # layers 0..6 -> A, 7..9 -> B
NJ = 8                         # 1024/128 blocks per stack
NG = 8                         # residue groups of 16
NCORES = 8
TOK = B_ * S_                  # 16384 tokens
TPC = TOK // NCORES            # 2048 tokens per core
ST_TOK = 512                   # supertile tokens
NSUP = TPC // ST_TOK           # 4 supertiles

F32 = mybir.dt.float32
F16 = mybir.dt.float16


# ---------------------------------------------------------------- host factor
def _apply_layers(h, twiddle, layers):
    T, nstacks, n = h.shape
    for i in layers:
        stride = 1 << i
        nblk = n // (2 * stride)
        hr = h.reshape(T, nstacks, nblk, 2, stride)
        t = twiddle[:, i].reshape(nstacks, nblk, stride, 2, 2)
        hr = np.einsum('kbpoi,Tkbip->Tkbop', t, hr)
        h = hr.reshape(T, nstacks, n)
    return h


def _factor_weights(twiddle, bias):
    tw = np.asarray(twiddle, np.float64)
    eye = np.broadcast_to(np.eye(N_)[:, None, :], (N_, NSTACKS, N_)).copy()
    hA = _apply_layers(eye, tw, range(SPLIT))
    A_full = hA.transpose(1, 2, 0)          # [k, out_feat, in_feat]
    hB = _apply_layers(eye, tw, range(SPLIT, LOG_N))
    B_full = hB.transpose(1, 2, 0)

    # At[k, j, c, m] = A[k,j][m, c]  (lhsT layout: [K=c, M=m]), with the
    # column (output-partition) order permuted m=16g+a -> p=8a+g so that
    # stage-A output partitions are (a, g)-indexed for the shuffle.
    At = np.empty((NSTACKS, NJ, 128, 128), np.float32)
    for j in range(NJ):
        blk = A_full[:, 128 * j:128 * (j + 1), 128 * j:128 * (j + 1)]
        At[:, j] = blk.transpose(0, 2, 1)
    At = At.reshape(NSTACKS, NJ, 128, 8, 16).transpose(0, 1, 2, 4, 3) \
           .reshape(NSTACKS, NJ, 128, 128)

    # Bmat[k, r, j', j] = B_full[k, 128j'+r, 128j+r]
    jj = 128 * np.arange(NJ)
    Bmat = np.empty((NSTACKS, 128, NJ, NJ))
    for r in range(128):
        Bmat[:, r] = B_full[:, jj[:, None] + r, jj[None, :] + r]

    # Bw[k, g, q=(8a+j), of=(16j'+a)] = Bmat[k, 16g+a, j', j]
    # lhsT for of-major stage B: out[of, t] = sum_q Bw[q, of] y_sa[q, t]
    Bw = np.zeros((NSTACKS, NG, 128, 128), np.float32)
    j16 = 16 * np.arange(NJ)
    j8 = np.arange(NJ)
    for k in range(NSTACKS):
        for g in range(NG):
            for a in range(16):
                Bw[k, g][np.ix_(8 * a + j8, j16 + a)] = Bmat[k, 16 * g + a].T

    # bias_col[k, g, p=(16j'+a)] = bias[1024k + 128j' + 16g + a]
    bias = np.asarray(bias, np.float64)
    bcol = np.empty((NSTACKS, NG, 128))
    for k in range(NSTACKS):
        for g in range(NG):
            for jp in range(NJ):
                bcol[k, g, 16 * jp:16 * jp + 16] = \
                    bias[1024 * k + 128 * jp + 16 * g + np.arange(16)]
    # device layout [128 p, NSTACKS*NG]
    bias_sb = np.ascontiguousarray(bcol.transpose(2, 0, 1).reshape(128, -1))
    return At, Bw.astype(np.float32), bias_sb.astype(np.float32)


# ---------------------------------------------------------------- device IR
def build_kernel():
    nc = bacc.Bacc()
    # x arrives feature-major: xf[c, NJ*t + ...] -> [128, NJ*TPC]
    xf_d = nc.dram_tensor("xf", [128, NJ * TPC], F16, kind="ExternalInput")
    At_d = nc.dram_tensor("At", [128, NSTACKS * NJ * 128], F16, kind="ExternalInput")
    Bw_d = nc.dram_tensor("Bw", [128, NSTACKS * NG * 128], F16, kind="ExternalInput")
    bias_d = nc.dram_tensor("biasc", [128, NSTACKS * NG], F32, kind="ExternalInput")
    # out rows = 1024k + 8p + g with p = 16j'+a; host unpermutes
    out_d = nc.dram_tensor("out", [NSTACKS * N_, TPC], F16, kind="ExternalOutput")

    ATW = NJ * 128       # At free size per stack
    BWW = NG * 128       # Bw free size per stack
    YPITCH = NJ * TPC    # y_ba free elems per partition

    with tile.TileContext(nc) as tc:
        with (
            tc.tile_pool(name="consts", bufs=1) as consts,
            tc.tile_pool(name="xfa", bufs=4) as xfa_p,
            tc.tile_pool(name="yba", bufs=2) as yba_p,
            tc.tile_pool(name="ysa", bufs=8) as ysa_p,
            tc.tile_pool(name="outb", bufs=10) as outb_p,
            tc.tile_pool(name="ps_y", bufs=5, space="PSUM") as psy_p,
            tc.tile_pool(name="ps_o", bufs=3, space="PSUM") as pso_p,
        ):
            At_sb = consts.tile([128, NSTACKS * ATW], F16)   # part=c, free=(k,j,m)
            Bw_sb = consts.tile([128, NSTACKS * BWW], F16)   # part=q, free=(k,g,of)
            bias_sb = consts.tile([128, NSTACKS * NG], F32)  # part=p, free=(k,g)
            warm_sb = consts.tile([128, 256], F16)
            nc.vector.memset(warm_sb, 0.0)

            # --- PE warmup: dependency-free K=128 accumulation chain so the
            # HAM clock-gate lifts the PE before real work arrives.
            warm_ps = pso_p.tile([128, ST_TOK], F32, tag="pso")
            for w in range(16):
                nc.tensor.matmul(warm_ps[:, 0:256], warm_sb[:, 0:128], warm_sb,
                                 start=(w == 0), stop=(w == 15),
                                 skip_group_check=True)

            x_fa = []
            for _xj in range(NJ // 2):
                xft = xfa_p.tile([128, 2 * TPC], F16, tag="xfa", name="xft")
                x_fa.append(xft)

            def load_x(jp):
                # load j-pair {2jp, 2jp+1}: 8-KiB descriptors
                nc.sync.dma_start(
                    out=x_fa[jp],
                    in_=bass.AP(tensor=xf_d, offset=2 * jp * TPC,
                                ap=[[NJ * TPC, 128], [1, 2 * TPC]]))

            nc.sync.dma_start(
                out=At_sb,
                in_=bass.AP(tensor=At_d, offset=0,
                            ap=[[NSTACKS * ATW, 128], [1, NSTACKS * ATW]]))
            for jp in range(NJ // 2):
                load_x(jp)
            nc.sync.dma_start(
                out=bias_sb,
                in_=bass.AP(tensor=bias_d, offset=0,
                            ap=[[NSTACKS * NG, 128], [1, NSTACKS * NG]]))
            nc.sync.dma_start(
                out=Bw_sb,
                in_=bass.AP(tensor=Bw_d, offset=0,
                            ap=[[NSTACKS * BWW, 128], [1, NSTACKS * BWW]]))

            y_ba = {}     # k -> full-stack stage-A output tile [128, NJ*TPC]
            y_sa = {}     # (k, g) -> shuffled tile [128, TPC]
            out_hf = {}   # (k, h) -> gelu output half tile [128, 4*TPC]

            def stage_a_step(k, i):
                # i in 0..31 enumerates (j, st) j-outer; one matmul + evac.
                j, st = i // NSUP, i % NSUP
                if i == 0:
                    y_ba[k] = yba_p.tile([128, NJ * TPC], F16, tag="yba", name="yba")
                yb = y_ba[k]
                yps = psy_p.tile([128, ST_TOK], F32, name="yps")
                nc.tensor.matmul(
                    yps,
                    At_sb[:, k * ATW + j * 128:k * ATW + (j + 1) * 128],
                    x_fa[j // 2][:, TPC * (j % 2) + ST_TOK * st:
                                 TPC * (j % 2) + ST_TOK * (st + 1)],
                    start=True, stop=True)
                dst = yb[:, TPC * j + ST_TOK * st:TPC * j + ST_TOK * (st + 1)]
                if k == 0 and i % 2 == 1:
                    nc.scalar.copy(dst, yps)
                else:
                    nc.vector.tensor_copy(dst, yps)

            def shuffle(k):
                # y_sa[(k,g)][q=8a+j, t] = y_ba[k][p=8a+g, j*TPC + t]
                yb = y_ba[k]
                for g in range(NG):
                    yt = ysa_p.tile([128, TPC], F16, tag="ysa", name="ysa")
                    y_sa[(k, g)] = yt
                    eng = nc.scalar if g in (1, 3) else nc.sync
                    eng.dma_start(
                        out=yt,
                        in_=bass.AP(tensor=yb.tensor,
                                    offset=yb.offset + g * YPITCH,
                                    ap=[[8 * YPITCH, 16], [TPC, NJ], [1, TPC]]))

            def stage_b_step(k, i):
                # i in 0..31 enumerates (g, st) g-outer; one matmul + gelu.
                g, st = i // NSUP, i % NSUP
                if st == 0:
                    out_hf[(k, g)] = outb_p.tile([128, TPC], F16, tag="outsb", name="osb")
                osb = out_hf[(k, g)]
                pso = pso_p.tile([128, ST_TOK], F32, tag="pso", name="pso")
                nc.tensor.matmul(
                    pso,
                    Bw_sb[:, (k * NG + g) * 128:(k * NG + g + 1) * 128],
                    y_sa[(k, g)][:, ST_TOK * st:ST_TOK * (st + 1)],
                    start=True, stop=True)
                nc.scalar.activation(
                    osb[:, ST_TOK * st:ST_TOK * (st + 1)],
                    pso, mybir.ActivationFunctionType.Gelu,
                    bias=bias_sb[:, k * NG + g:k * NG + g + 1])
                if st == NSUP - 1:
                    # store rows 1024k + 8p + g as soon as g's supertiles done
                    nc.gpsimd.dma_start(
                        out=bass.AP(tensor=out_d,
                                    offset=(N_ * k + g) * TPC,
                                    ap=[[8 * TPC, 128], [1, TPC]]),
                        in_=osb)

            # ---- software-pipelined emission: stage B(k) interleaves with
            # front-loaded stage A(k+1) (3 A-steps per 2 B-steps) so A(k+1)
            # and its shuffle complete before B(k+1) needs them.
            for i in range(32):
                stage_a_step(0, i)
            for k in range(NSTACKS):
                shuffle(k)
                a_ptr = 0
                for i in range(32):
                    stage_b_step(k, i)
                    if k + 1 < NSTACKS:
                        tgt = min(32, ((i + 1) * 3 + 1) // 2)
                        while a_ptr < tgt:
                            stage_a_step(k + 1, a_ptr)
                            a_ptr += 1

    nc.finalize()
    return nc


_NC_CACHE = None
_PERM_CACHE = None


def _out_perm():
    # HBM row = 1024k + 8p + g with p = 16j'+a  <->  feat = 1024k+128j'+16g+a
    global _PERM_CACHE
    if _PERM_CACHE is None:
        feat = np.arange(NSTACKS * N_)
        k, r = feat // N_, feat % N_
        jp, rem = r // 128, r % 128
        g, a = rem // 16, rem % 16
        _PERM_CACHE = N_ * k + 8 * (16 * jp + a) + g
    return _PERM_CACHE


def kernel(hidden_states, twiddle, bias):
    global _NC_CACHE
    x = np.ascontiguousarray(np.asarray(hidden_states, np.float32)).reshape(TOK, N_)
    At, Bw, bias_sb = _factor_weights(twiddle, bias)

    if _NC_CACHE is None:
        _NC_CACHE = build_kernel()
    nc = _NC_CACHE

    # partition-major device layouts: At[c, (k,j,m)], Bw[q, (k,g,of)]
    At16 = np.ascontiguousarray(
        At.astype(np.float16).transpose(2, 0, 1, 3).reshape(128, -1))
    Bw16 = np.ascontiguousarray(
        Bw.astype(np.float16).transpose(2, 0, 1, 3).reshape(128, -1))
    x16 = x.astype(np.float16)

    in_maps = []
    for i in range(NCORES):
        # feature-major per-core layout: xf[c, TPC*j + t] = x[i*TPC+t, 128j+c]
        xs = x16[i * TPC:(i + 1) * TPC].reshape(TPC, NJ, 128)
        xf = np.ascontiguousarray(xs.transpose(2, 1, 0)).reshape(128, NJ * TPC)
        in_maps.append({
            "xf": xf, "At": At16, "Bw": Bw16, "biasc": bias_sb,
        })
    res = bass_utils.run_bass_kernel_spmd(nc, in_maps, core_ids=list(range(NCORES)))
    global LAST_RESULT
    LAST_RESULT = res
    perm = _out_perm()
    out = np.empty((TOK, NSTACKS * N_), np.float32)
    for i in range(NCORES):
        buf = res.results[i]["out"]          # [4096 rows, TPC] f16
        out[i * TPC:(i + 1) * TPC] = buf[perm].T.astype(np.float32)
    return out.reshape(B_, S_, NSTACKS * N_)


LAST_RESULT = None


if __name__ == "__main__":
    rng = np.random.default_rng(0)
    h = rng.standard_normal((B_, S_, N_), dtype=np.float32)
    tw = (rng.standard_normal((NSTACKS, LOG_N, N_ // 2, 2, 2)) * 2 ** -0.5).astype(np.float32)
    b = rng.standard_normal(NSTACKS * N_).astype(np.float32)
    out = kernel(h, tw, b)
    print("out", out.shape, out.dtype, np.abs(out).max())


# revision 20
# speedup vs baseline: 1.0251x; 1.0251x over previous
"""Trainium2 Bass kernel for nn_Bfly_BertIntermediate (butterfly MLP + bias + gelu).

Algorithm ("Monarch" factorization of the 10-layer butterfly over N=1024):
  layers 0..6  (strides 1..64) == block-diagonal A: 8 blocks of 128x128 per stack
  layers 7..9  (strides 128..512) == per-residue-128 mixing B: 8x8 over block idx j

v2 device pipeline per core (2048 tokens, data-parallel over 8 cores):
  x arrives feature-major f16 -> stage-A matmuls (feature-major psum out)
  -> DVE/Pool alternating cast-evac f32->f16 into y_ba (full-stack buffer)
  -> one 4-KiB-descriptor shuffle DMA per (stack, residue-group) y_ba->y_sa
  -> stage-B of-major matmuls (out = Bw.T @ y_sa, features on partitions)
  -> ACT gelu with fused per-partition bias, contiguous f16 writes
  -> 16-KiB-descriptor SWDGE stores to a permuted feature-major HBM layout
  (host unpermutes/transposes during unshard).

A/Bw/bias are a tiny host-side repacking of the twiddle weights (~0.1 GFLOP).
"""
import numpy as np

import concourse.bass as bass
import concourse.mybir as mybir
import concourse.tile as tile
from concourse import bacc, bass_utils

# problem shapes (hardcoded per harness contract)
B_, S_, N_ = 4, 4096, 1024
NSTACKS, LOG_N = 4, 10
SPLIT = 7                      # layers 0..6 -> A, 7..9 -> B
NJ = 8                         # 1024/128 blocks per stack
NG = 8                         # residue groups of 16
NCORES = 8
TOK = B_ * S_                  # 16384 tokens
TPC = TOK // NCORES            # 2048 tokens per core
ST_TOK = 512                   # supertile tokens
NSUP = TPC // ST_TOK           # 4 supertiles

F32 = mybir.dt.float32
F16 = mybir.dt.float16


# ---------------------------------------------------------------- host factor
def _apply_layers(h, twiddle, layers):
    T, nstacks, n = h.shape
    for i in layers:
        stride = 1 << i
        nblk = n // (2 * stride)
        hr = h.reshape(T, nstacks, nblk, 2, stride)
        t = twiddle[:, i].reshape(nstacks, nblk, stride, 2, 2)
        hr = np.einsum('kbpoi,Tkbip->Tkbop', t, hr)
        h = hr.reshape(T, nstacks, n)
    return h


def _factor_weights(twiddle, bias):
    tw = np.asarray(twiddle, np.float64)
    eye = np.broadcast_to(np.eye(N_)[:, None, :], (N_, NSTACKS, N_)).copy()
    hA = _apply_layers(eye, tw, range(SPLIT))
    A_full = hA.transpose(1, 2, 0)          # [k, out_feat, in_feat]
    hB = _apply_layers(eye, tw, range(SPLIT, LOG_N))
    B_full = hB.transpose(1, 2, 0)

    # At[k, j, c, m] = A[k,j][m, c]  (lhsT layout: [K=c, M=m]), with the
    # column (output-partition) order permuted m=16g+a -> p=8a+g so that
    # stage-A output partitions are (a, g)-indexed for the shuffle.
    At = np.empty((NSTACKS, NJ, 128, 128), np.float32)
    for j in range(NJ):
        blk = A_full[:, 128 * j:128 * (j + 1), 128 * j:128 * (j + 1)]
        At[:, j] = blk.transpose(0, 2, 1)
    At = At.reshape(NSTACKS, NJ, 128, 8, 16).transpose(0, 1, 2, 4, 3) \
           .reshape(NSTACKS, NJ, 128, 128)

    # Bmat[k, r, j', j] = B_full[k, 128j'+r, 128j+r]
    jj = 128 * np.arange(NJ)
    Bmat = np.empty((NSTACKS, 128, NJ, NJ))
    for r in range(128):
        Bmat[:, r] = B_full[:, jj[:, None] + r, jj[None, :] + r]

    # Bw[k, g, q=(8a+j), of=(16j'+a)] = Bmat[k, 16g+a, j', j]
    # lhsT for of-major stage B: out[of, t] = sum_q Bw[q, of] y_sa[q, t]
    Bw = np.zeros((NSTACKS, NG, 128, 128), np.float32)
    j16 = 16 * np.arange(NJ)
    j8 = np.arange(NJ)
    for k in range(NSTACKS):
        for g in range(NG):
            for a in range(16):
                Bw[k, g][np.ix_(8 * a + j8, j16 + a)] = Bmat[k, 16 * g + a].T

    # bias_col[k, g, p=(16j'+a)] = bias[1024k + 128j' + 16g + a]
    bias = np.asarray(bias, np.float64)
    bcol = np.empty((NSTACKS, NG, 128))
    for k in range(NSTACKS):
        for g in range(NG):
            for jp in range(NJ):
                bcol[k, g, 16 * jp:16 * jp + 16] = \
                    bias[1024 * k + 128 * jp + 16 * g + np.arange(16)]
    # device layout [128 p, NSTACKS*NG]
    bias_sb = np.ascontiguousarray(bcol.transpose(2, 0, 1).reshape(128, -1))
    return At, Bw.astype(np.float32), bias_sb.astype(np.float32)


# ---------------------------------------------------------------- device IR
def build_kernel():
    nc = bacc.Bacc()
    # x arrives feature-major: xf[c, NJ*t + ...] -> [128, NJ*TPC]
    xf_d = nc.dram_tensor("xf", [128, NJ * TPC], F16, kind="ExternalInput")
    At_d = nc.dram_tensor("At", [128, NSTACKS * NJ * 128], F16, kind="ExternalInput")
    Bw_d = nc.dram_tensor("Bw", [128, NSTACKS * NG * 128], F16, kind="ExternalInput")
    bias_d = nc.dram_tensor("biasc", [128, NSTACKS * NG], F32, kind="ExternalInput")
    # out rows = 1024k + 8p + g with p = 16j'+a; host unpermutes
    out_d = nc.dram_tensor("out", [NSTACKS * N_, TPC], F16, kind="ExternalOutput")

    ATW = NJ * 128       # At free size per stack
    BWW = NG * 128       # Bw free size per stack
    YPITCH = NJ * TPC    # y_ba free elems per partition

    with tile.TileContext(nc) as tc:
        with (
            tc.tile_pool(name="consts", bufs=1) as consts,
            tc.tile_pool(name="xfa", bufs=4) as xfa_p,
            tc.tile_pool(name="yba", bufs=2) as yba_p,
            tc.tile_pool(name="ysa", bufs=8) as ysa_p,
            tc.tile_pool(name="outb", bufs=10) as outb_p,
            tc.tile_pool(name="ps_y", bufs=5, space="PSUM") as psy_p,
            tc.tile_pool(name="ps_o", bufs=3, space="PSUM") as pso_p,
        ):
            At_sb = consts.tile([128, NSTACKS * ATW], F16)   # part=c, free=(k,j,m)
            Bw_sb = consts.tile([128, NSTACKS * BWW], F16)   # part=q, free=(k,g,of)
            bias_sb = consts.tile([128, NSTACKS * NG], F32)  # part=p, free=(k,g)
            warm_sb = consts.tile([128, 256], F16)
            nc.vector.memset(warm_sb, 0.0)

            # --- PE warmup: dependency-free K=128 accumulation chain so the
            # HAM clock-gate lifts the PE before real work arrives.
            warm_ps = pso_p.tile([128, ST_TOK], F32, tag="pso")
            for w in range(16):
                nc.tensor.matmul(warm_ps[:, 0:256], warm_sb[:, 0:128], warm_sb,
                                 start=(w == 0), stop=(w == 15),
                                 skip_group_check=True)

            x_fa = []
            for _xj in range(NJ // 2):
                xft = xfa_p.tile([128, 2 * TPC], F16, tag="xfa", name="xft")
                x_fa.append(xft)

            def load_x(jp):
                # load j-pair {2jp, 2jp+1}: 8-KiB descriptors
                nc.sync.dma_start(
                    out=x_fa[jp],
                    in_=bass.AP(tensor=xf_d, offset=2 * jp * TPC,
                                ap=[[NJ * TPC, 128], [1, 2 * TPC]]))

            nc.sync.dma_start(
                out=At_sb,
                in_=bass.AP(tensor=At_d, offset=0,
                            ap=[[NSTACKS * ATW, 128], [1, NSTACKS * ATW]]))
            for jp in range(NJ // 2):
                load_x(jp)
            nc.sync.dma_start(
                out=bias_sb,
                in_=bass.AP(tensor=bias_d, offset=0,
                            ap=[[NSTACKS * NG, 128], [1, NSTACKS * NG]]))
            nc.sync.dma_start(
                out=Bw_sb,
                in_=bass.AP(tensor=Bw_d, offset=0,
                            ap=[[NSTACKS * BWW, 128], [1, NSTACKS * BWW]]))

            y_ba = {}     # k -> full-stack stage-A output tile [128, NJ*TPC]
            y_sa = {}     # (k, g) -> shuffled tile [128, TPC]
            out_hf = {}   # (k, h) -> gelu output half tile [128, 4*TPC]

            def stage_a_step(k, i):
                # i in 0..31 enumerates (j, st) j-outer; one matmul + evac.
                j, st = i // NSUP, i % NSUP
                if i == 0:
                    y_ba[k] = yba_p.tile([128, NJ * TPC], F16, tag="yba", name="yba")
                yb = y_ba[k]
                yps = psy_p.tile([128, ST_TOK], F32, name="yps")
                nc.tensor.matmul(
                    yps,
                    At_sb[:, k * ATW + j * 128:k * ATW + (j + 1) * 128],
                    x_fa[j // 2][:, TPC * (j % 2) + ST_TOK * st:
                                 TPC * (j % 2) + ST_TOK * (st + 1)],
                    start=True, stop=True)
                dst = yb[:, TPC * j + ST_TOK * st:TPC * j + ST_TOK * (st + 1)]
                if k == 0 and i % 2 == 1:
                    nc.scalar.copy(dst, yps)
                else:
                    nc.vector.tensor_copy(dst, yps)

            def shuffle(k):
                # y_sa[(k,g)][q=8a+j, t] = y_ba[k][p=8a+g, j*TPC + t]
                yb = y_ba[k]
                for g in range(NG):
                    yt = ysa_p.tile([128, TPC], F16, tag="ysa", name="ysa")
                    y_sa[(k, g)] = yt
                    eng = nc.sync if g % 2 == 0 else nc.scalar
                    eng.dma_start(
                        out=yt,
                        in_=bass.AP(tensor=yb.tensor,
                                    offset=yb.offset + g * YPITCH,
                                    ap=[[8 * YPITCH, 16], [TPC, NJ], [1, TPC]]))

            def stage_b_step(k, i):
                # i in 0..31 enumerates (g, st) g-outer; one matmul + gelu.
                g, st = i // NSUP, i % NSUP
                if st == 0:
                    out_hf[(k, g)] = outb_p.tile([128, TPC], F16, tag="outsb", name="osb")
                osb = out_hf[(k, g)]
                pso = pso_p.tile([128, ST_TOK], F32, tag="pso", name="pso")
                nc.tensor.matmul(
                    pso,
                    Bw_sb[:, (k * NG + g) * 128:(k * NG + g + 1) * 128],
                    y_sa[(k, g)][:, ST_TOK * st:ST_TOK * (st + 1)],
                    start=True, stop=True)
                nc.scalar.activation(
                    osb[:, ST_TOK * st:ST_TOK * (st + 1)],
                    pso, mybir.ActivationFunctionType.Gelu,
                    bias=bias_sb[:, k * NG + g:k * NG + g + 1])
                if st == NSUP - 1:
                    # store rows 1024k + 8p + g as soon as g's supertiles done
                    nc.gpsimd.dma_start(
                        out=bass.AP(tensor=out_d,
                                    offset=(N_ * k + g) * TPC,
                                    ap=[[8 * TPC, 128], [1, TPC]]),
                        in_=osb)

            # ---- software-pipelined emission: stage B(k) interleaves with
            # front-loaded stage A(k+1) (3 A-steps per 2 B-steps) so A(k+1)
            # and its shuffle complete before B(k+1) needs them.
            for i in range(32):
                stage_a_step(0, i)
            for k in range(NSTACKS):
                shuffle(k)
                a_ptr = 0
                for i in range(32):
                    stage_b_step(k, i)
                    if k + 1 < NSTACKS:
                        tgt = min(32, ((i + 1) * 3 + 1) // 2)
                        while a_ptr < tgt:
                            stage_a_step(k + 1, a_ptr)
                            a_ptr += 1

    nc.finalize()
    return nc


_NC_CACHE = None
_PERM_CACHE = None


def _out_perm():
    # HBM row = 1024k + 8p + g with p = 16j'+a  <->  feat = 1024k+128j'+16g+a
    global _PERM_CACHE
    if _PERM_CACHE is None:
        feat = np.arange(NSTACKS * N_)
        k, r = feat // N_, feat % N_
        jp, rem = r // 128, r % 128
        g, a = rem // 16, rem % 16
        _PERM_CACHE = N_ * k + 8 * (16 * jp + a) + g
    return _PERM_CACHE


def kernel(hidden_states, twiddle, bias):
    global _NC_CACHE
    x = np.ascontiguousarray(np.asarray(hidden_states, np.float32)).reshape(TOK, N_)
    At, Bw, bias_sb = _factor_weights(twiddle, bias)

    if _NC_CACHE is None:
        _NC_CACHE = build_kernel()
    nc = _NC_CACHE

    # partition-major device layouts: At[c, (k,j,m)], Bw[q, (k,g,of)]
    At16 = np.ascontiguousarray(
        At.astype(np.float16).transpose(2, 0, 1, 3).reshape(128, -1))
    Bw16 = np.ascontiguousarray(
        Bw.astype(np.float16).transpose(2, 0, 1, 3).reshape(128, -1))
    x16 = x.astype(np.float16)

    in_maps = []
    for i in range(NCORES):
        # feature-major per-core layout: xf[c, TPC*j + t] = x[i*TPC+t, 128j+c]
        xs = x16[i * TPC:(i + 1) * TPC].reshape(TPC, NJ, 128)
        xf = np.ascontiguousarray(xs.transpose(2, 1, 0)).reshape(128, NJ * TPC)
        in_maps.append({
            "xf": xf, "At": At16, "Bw": Bw16, "biasc": bias_sb,
        })
    res = bass_utils.run_bass_kernel_spmd(nc, in_maps, core_ids=list(range(NCORES)))
    global LAST_RESULT
    LAST_RESULT = res
    perm = _out_perm()
    out = np.empty((TOK, NSTACKS * N_), np.float32)
    for i in range(NCORES):
        buf = res.results[i]["out"]          # [4096 rows, TPC] f16
        out[i * TPC:(i + 1) * TPC] = buf[perm].T.astype(np.float32)
    return out.reshape(B_, S_, NSTACKS * N_)


LAST_RESULT = None


if __name__ == "__main__":
    rng = np.random.default_rng(0)
    h = rng.standard_normal((B_, S_, N_), dtype=np.float32)
    tw = (rng.standard_normal((NSTACKS, LOG_N, N_ // 2, 2, 2)) * 2 ** -0.5).astype(np.float32)
    b = rng.standard_normal(NSTACKS * N_).astype(np.float32)
    out = kernel(h, tw, b)
    print("out", out.shape, out.dtype, np.abs(out).max())
